# revision 37
# baseline (speedup 1.0000x reference)
"""BidirectionalSAGE (2-layer bidir GraphSAGE + mean-pool + linear head) on 8
Trainium2 NeuronCores.

Strategy (node sharding, edge-cut by dst):
- Nodes split into 8 contiguous shards of 12500 (padded to 12800 = 100 panels
  of 128). Each core aggregates all edges whose dst is in its shard, for both
  edge directions.
- Edge features gathered with SWDGE dma_gather (int16 idxs, 4 address chunks)
  from a packed table [102400, 128] bf16 of rows [bf16(v) | bf16(v-bf16(v))]
  (hi|lo split). Trailing pad idxs are -1 so the Q7 descriptor generator
  skips them.
- Segment-sum = one-hot matmul: psum[featpack, dst] += gt.T @ S, with
  S[e, j] = (local_dst[e] == j) built 8 subtiles at a time by one
  tensor_tensor is_equal over broadcast APs (vector engine).
- PSUM drain = one tensor_tensor multiply per 4-panel bank that folds in
  0.5/deg and emits bf16 mean tiles directly consumable as GEMM rhs.
- GEMMs are bf16 with hi|lo-stacked weights [Wl;Wl] so the hi+lo fold happens
  inside the matmul contraction; rhs is 512 wide (4 panels per matmul).
- relu/bias on the ACT engine; h is kept feature-major (hT) and PE-transposed
  per panel into row-major packed rows for the next layer's gather table /
  the pooling matmul.
- h (layer-1 out) is AllGathered across cores as the layer-2 gather table.
- Per-graph mean-pool via one-hot matmul against graph ids; each core emits
  (sums_partial * inv_cnt) @ predW.T + pred_b/8, summed on the host.
"""
import os
import sys
import types
import contextlib
import ctypes

import numpy as np

_SO_PATH = "/opt/axon/libaxon_pjrt.so"


def _install_ntff_shim():
    if "antenv.axon_hooks" in sys.modules:
        return
    try:
        lib = ctypes.CDLL(_SO_PATH)
    except OSError:
        return
    if not hasattr(lib, "axon_start_nrt_profile"):
        hook = None
    else:
        lib.axon_start_nrt_profile.argtypes = [
            ctypes.POINTER(ctypes.c_int64),
            ctypes.c_size_t,
        ]
        lib.axon_start_nrt_profile.restype = ctypes.c_int64
        lib.axon_stop_nrt_profile.argtypes = [ctypes.c_char_p]
        lib.axon_stop_nrt_profile.restype = ctypes.c_int64

        @contextlib.contextmanager
        def hook(output_dir, device_ids):
            import jax

            jax.devices()
            if device_ids:
                ids = (ctypes.c_int64 * len(device_ids))(*device_ids)
                rc = lib.axon_start_nrt_profile(ids, len(device_ids))
            else:
                rc = lib.axon_start_nrt_profile(None, 0)
            if rc != 0:
                raise RuntimeError(f"axon_start_nrt_profile rc={rc}")
            try:
                yield
            finally:
                n = lib.axon_stop_nrt_profile(str(output_dir).encode())
                print(f"profile: {n} file(s) in {output_dir}", file=sys.stderr)

    mod = types.ModuleType("antenv.axon_hooks")
    mod.get_axon_ntff_profile_hook = lambda: hook
    mod.set_axon_ntff_profile_hook = lambda h: None
    sys.modules["antenv.axon_hooks"] = mod


_install_ntff_shim()

import concourse.bass as bass
import concourse.bacc as bacc
import concourse.mybir as mybir
import concourse.tile as tile
from concourse.bass_utils import run_bass_kernel_spmd
from concourse.masks import make_identity

# problem constants (hardcoded per spec)
N = 100000
E = 1000000
D = 64
G = 128
OUT = 16
NCORES = 8
SHARD = 12500          # real nodes per core
PANEL = 128
NPANEL = 100           # padded panels per shard
SHARD_PAD = NPANEL * PANEL   # 12800
TBL_ROWS = SHARD_PAD * NCORES  # 102400 packed-table rows
NCHUNK = 4
CHUNK_ROWS = TBL_ROWS // NCHUNK  # 25600 (< 32767: int16-addressable)
PANELS_PER_PASS = 20
NPASS = NPANEL // PANELS_PER_PASS  # 5
CALL_MAX = int(os.environ.get("SAGE_CALL_MAX", "2048"))  # dma_gather idxs per call
SINGLE_PACKET = os.environ.get("SAGE_SP", "0") == "1"
CELL_ROUND = int(os.environ.get("SAGE_CELL_ROUND", "16"))  # cell padding granule
SBATCH = 8             # subtiles per one-hot build

LAST_RESULTS = None


def pack_hilo(v):
    import ml_dtypes

    hi = v.astype(ml_dtypes.bfloat16)
    lo = (v - hi.astype(np.float32)).astype(ml_dtypes.bfloat16)
    return np.concatenate([hi, lo], axis=1)  # [rows, 2D]


QROWS = SHARD_PAD // NCHUNK  # 3200 rows per shard quarter


def tbl_row(src):
    """node id -> packed-table row.

    Quarter-major layout: table chunk q holds quarter q of EVERY shard, so
    the inter-layer AllGather can run as 4 slice-collectives, each unblocking
    one gather chunk of layer 1.
    """
    core = src // SHARD
    loc = src % SHARD
    q = loc // QROWS
    return q * (NCORES * QROWS) + core * QROWS + (loc - q * QROWS)


def wrap_idx16(idxs):
    """int idx stream (len % 128 == 0) -> [128, len/16] int16 wrapped layout."""
    n = len(idxs)
    s = n // 16
    blk = idxs.reshape(s, 16).T.astype(np.int16)  # [16, s]
    return np.tile(blk, (8, 1))  # replicate to all 8 Q7 groups -> [128, s]


def build_dir_plan(src, dst):
    """One edge direction. Returns uniform structure + per-core data arrays."""
    core = dst // SHARD
    d_loc = dst - core * SHARD
    panel = d_loc // PANEL
    ld = d_loc % PANEL
    row = tbl_row(src)
    chunk = row // CHUNK_ROWS
    q = row - chunk * CHUNK_ROWS  # chunk-local idx (< 25600)

    pas = panel // PANELS_PER_PASS

    # counts[core, pass, chunk, panel]
    counts = np.zeros((NCORES, NPASS, NCHUNK, NPANEL), np.int64)
    np.add.at(counts, (core, pas, chunk, panel), 1)
    cmax = counts.max(axis=0)  # [npass, chunk, panel]
    R = CELL_ROUND
    C = ((cmax + R - 1) // R) * R  # uniform padded counts
    valid = np.zeros((NPASS, NCHUNK, NPANEL), bool)
    for p in range(NPASS):
        valid[p, :, p * PANELS_PER_PASS:(p + 1) * PANELS_PER_PASS] = True
    C[~valid] = 0
    # every panel needs >=1 matmul (psum init) in its own pass
    for p in range(NPASS):
        for pan in range(p * PANELS_PER_PASS,
                         min((p + 1) * PANELS_PER_PASS, NPANEL)):
            if C[p, :, pan].sum() == 0:
                C[p, 0, pan] = R
    # per (pass, chunk) streams must stay 128-aligned for the 2D ld layout
    for p in range(NPASS):
        for ch in range(NCHUNK):
            rem = int(C[p, ch].sum()) % 128
            if rem:
                pan = np.flatnonzero(C[p, ch])[-1]
                C[p, ch, pan] += 128 - rem

    order = np.lexsort((panel, chunk, pas, core))
    q_s, ld_s = q[order], ld[order]
    core_s, pas_s, chunk_s, panel_s = core[order], pas[order], chunk[order], panel[order]

    # build padded per-core streams
    total = int(C.sum())  # rows per core (uniform)
    idx_all = np.zeros((NCORES, total), np.int64)
    ld_all = np.full((NCORES, total), -1.0, np.float32)

    flat_off = np.concatenate([[0], np.cumsum(C.ravel())[:-1]]).reshape(
        NPASS, NCHUNK, NPANEL)

    grp = ((core_s * NPASS + pas_s) * NCHUNK + chunk_s) * NPANEL + panel_s
    first = np.concatenate([[0], np.flatnonzero(np.diff(grp)) + 1])
    sizes = np.diff(np.concatenate([first, [len(grp)]]))
    rank = np.arange(len(grp)) - np.repeat(first, sizes)

    pos = flat_off[pas_s, chunk_s, panel_s] + rank
    idx_all[core_s, pos] = q_s
    ld_all[core_s, pos] = ld_s

    # structure: per (pass, chunk): row count and gather-call split
    struct = []
    for p in range(NPASS):
        chunks = []
        for ch in range(NCHUNK):
            rows = int(C[p, ch].sum())
            calls = []
            r = rows
            while r > 0:
                c = min(r, CALL_MAX)
                calls.append(c)
                r -= c
            chunks.append((rows, calls))
        struct.append(chunks)

    # trailing-pad trim: within each gather call, pad idxs at the very end
    # are set to -1 so the Q7 SWDGE skips those descriptors entirely.
    if os.environ.get("SAGE_TRIM", "0") == "1":
        for c in range(NCORES):
            row_off = 0
            for p in range(NPASS):
                for ch in range(NCHUNK):
                    rows, calls = struct[p][ch]
                    r0 = row_off
                    for ni in calls:
                        seg_ld = ld_all[c, r0:r0 + ni]
                        k = ni
                        while k > 0 and seg_ld[k - 1] < 0:
                            k -= 1
                        idx_all[c, r0 + k:r0 + ni] = -1
                        r0 += ni
                    row_off += rows

    idx_wrapped = np.stack([wrap_idx16(idx_all[c]) for c in range(NCORES)])

    # split streams: per column, the first cell's lanes go to ld_lo; a second
    # cell sharing the column goes to a compacted ld_hi column. All matmuls
    # then run K=128 at partition base 0.
    totcols = total // 128
    ld_lo = np.full((NCORES, totcols, 128), -1.0, np.float32)
    ld_hi_cols = []           # list of [NCORES, 128] arrays
    lo_pan = {}               # (p, ch) -> list of panel per column
    hi_seg = {}               # (p, ch) -> list of (col, pan, hglob)
    nhi_off = [0] * (NPASS + 1)
    off0 = 0
    colg = 0
    for p in range(NPASS):
        for ch in range(NCHUNK):
            ncols = int(C[p, ch].sum()) // 128
            lo_list = [None] * ncols
            hi_list = []
            pos = 0
            for pan in range(p * PANELS_PER_PASS, (p + 1) * PANELS_PER_PASS):
                n = int(C[p, ch, pan])
                while n > 0:
                    col, l0 = pos // 128, pos % 128
                    nl = min(128 - l0, n)
                    vals = ld_all[:, off0 + col * 128 + l0:
                                  off0 + col * 128 + l0 + nl]
                    if l0 == 0:
                        lo_list[col] = pan
                        ld_lo[:, colg + col, :nl] = vals
                    else:
                        hc = np.full((NCORES, 128), -1.0, np.float32)
                        hc[:, l0:l0 + nl] = vals
                        hi_list.append((col, pan, len(ld_hi_cols)))
                        ld_hi_cols.append(hc)
                    pos += nl
                    n -= nl
            lo_pan[(p, ch)] = lo_list
            hi_seg[(p, ch)] = hi_list
            off0 += ncols * 128
            colg += ncols
        nhi_off[p + 1] = len(ld_hi_cols)

    ld_t = np.ascontiguousarray(ld_lo.transpose(0, 2, 1), np.float32)
    if ld_hi_cols:
        ldh = np.stack(ld_hi_cols, axis=1)  # [NCORES, nhi, 128]
        ldh_t = np.ascontiguousarray(ldh.transpose(0, 2, 1), np.float32)
    else:
        ldh_t = np.zeros((NCORES, 128, 1), np.float32)
    return {
        "C": C, "struct": struct, "total": total,
        "idx": idx_wrapped, "ld": ld_t, "ldh": ldh_t,
        "lo_pan": lo_pan, "hi_seg": hi_seg, "nhi_off": nhi_off,
    }


def build_bass(plans, weights):
    nc = bacc.Bacc("TRN2", target_bir_lowering=False, debug=False,
                   num_devices=NCORES, num_swdge_queues=4)
    f32, bf16, i16 = mybir.dt.float32, mybir.dt.bfloat16, mybir.dt.int16

    # ---- dram inputs ----
    x_pack_d = nc.dram_tensor("x_pack", [TBL_ROWS, 2 * D], bf16, kind="ExternalInput")
    iota_d = nc.dram_tensor("iota", [128, 128], f32, kind="ExternalInput")
    ins = {}
    for dname in ("f", "b"):
        p = plans[dname]
        ins[dname] = {
            "idx": nc.dram_tensor(f"idx_{dname}", list(p["idx"].shape[1:]), i16,
                                  kind="ExternalInput"),
            "ld": nc.dram_tensor(f"ld_{dname}", list(p["ld"].shape[1:]), f32,
                                 kind="ExternalInput"),
            "ldh": nc.dram_tensor(f"ldh_{dname}", list(p["ldh"].shape[1:]), f32,
                                  kind="ExternalInput"),
            "ivd": nc.dram_tensor(f"ivd_{dname}", [128, SHARD_PAD], bf16,
                                  kind="ExternalInput"),
        }
    gid_d = nc.dram_tensor("gid", [128, NPANEL], f32, kind="ExternalInput")
    xT_d = nc.dram_tensor("xT", [128, SHARD_PAD], bf16, kind="ExternalInput")
    icnt_d = nc.dram_tensor("icnt", [128, G], bf16, kind="ExternalInput")
    w_d = {}
    for k, v in weights.items():
        dt = f32 if k.endswith("bias") else bf16
        w_d[k] = nc.dram_tensor(k, list(v.shape), dt, kind="ExternalInput")
    out_d = nc.dram_tensor("out", [OUT, G], f32, kind="ExternalOutput")

    with tile.TileContext(nc) as tc:
        with tc.tile_pool(name="const", bufs=1) as cp, \
             tc.tile_pool(name="idxp", bufs=2) as idxp, \
             tc.tile_pool(name="ldp", bufs=2) as ldp, \
             tc.tile_pool(name="gp", bufs=8) as gp, \
             tc.tile_pool(name="sbp", bufs=6) as sbp, \
             tc.tile_pool(name="mp", bufs=1) as mp, \
             tc.tile_pool(name="up", bufs=4) as up, \
             tc.tile_pool(name="hp", bufs=1) as hp, \
             tc.tile_pool(name="ivp", bufs=2) as ivp, \
             tc.tile_pool(name="ps", bufs=1, space="PSUM") as ps, \
             tc.tile_pool(name="psu", bufs=1, space="PSUM") as psu, \
             tc.tile_pool(name="pst", bufs=1, space="PSUM") as pst, \
             tc.tile_pool(name="dram", bufs=1, space="DRAM") as dp:

            qrr = [0]
            iota_t = cp.tile([128, 128], f32)
            nc.sync.dma_start(out=iota_t[:], in_=iota_d[:])
            ident = cp.tile([128, 128], bf16)
            make_identity(nc, ident[:])

            wt = {}
            for k, v in weights.items():
                dt = f32 if k.endswith("bias") else bf16
                t = cp.tile(list(v.shape), dt, tag=f"w_{k}")
                nc.sync.dma_start(out=t[:], in_=w_d[k][:])
                wt[k] = t

            gid_t = cp.tile([128, NPANEL], f32)
            nc.sync.dma_start(out=gid_t[:], in_=gid_d[:])

            # persistent sbuf: transposed packed features of own shard
            xT_own = hp.tile([128, SHARD_PAD], bf16, tag="xT_own")
            nc.sync.dma_start(out=xT_own[:], in_=xT_d[:])
            # h keeps only the bf16 hi half; lo rows stay zero
            hT_own = hp.tile([128, SHARD_PAD], bf16, tag="hT_own")
            nc.vector.memset(hT_own[D:, :], 0.0)

            # dram bounces for allgather
            h_bounce = dp.tile([SHARD_PAD, 2 * D], bf16)
            h_table = dp.tile([TBL_ROWS, 2 * D], bf16)

            # pooled psum accumulator (layer-2 only)
            pool_ps = psu.tile([128, G], f32, tag="pool", name="poolbank")

            def seg_pass(layer, p, table_ap, mean_tiles):
                """Segment-sum one pass (both directions) into mean tiles."""
                p0 = p * PANELS_PER_PASS
                np_this = min(PANELS_PER_PASS, NPANEL - p0)
                banks = [ps.tile([128, 512], f32, tag=f"seg{i}",
                                 name=f"segbank{i}") for i in range(5)]

                for dname in ("f", "b"):
                    pl = plans[dname]
                    dio = ins[dname]
                    C = pl["C"]
                    sub_off = int(C[:p].sum()) // 128
                    row_off = int(C[:p].sum())
                    n_rows = int(C[p].sum())
                    n_sub = n_rows // 128

                    ivd_t = ivp.tile([128, np_this * 128], bf16, tag=f"ivd{dname}")
                    nc.sync.dma_start(
                        out=ivd_t[:],
                        in_=dio["ivd"][:, p0 * 128:p0 * 128 + np_this * 128])
                    ld_t = ldp.tile([128, n_sub], f32, tag=f"ld{dname}")
                    nc.sync.dma_start(
                        out=ld_t[:], in_=dio["ld"][:, sub_off:sub_off + n_sub])


                    # one-hot tiles, 8 subtiles per build
                    stiles = []
                    for b0 in range(0, n_sub, SBATCH):
                        nb = min(SBATCH, n_sub - b0)
                        s8 = sbp.tile([128, SBATCH, 128], bf16, tag=f"s{dname}")
                        nc.vector.tensor_tensor(
                            out=s8[:, :nb, :],
                            in0=ld_t[:, b0:b0 + nb, None].to_broadcast(
                                [128, nb, 128]),
                            in1=iota_t[:, None, :].to_broadcast([128, nb, 128]),
                            op=mybir.AluOpType.is_equal)
                        stiles.append(s8)
                    # one-hot tiles for split-column second cells
                    h0, h1 = pl["nhi_off"][p], pl["nhi_off"][p + 1]
                    n_hi = h1 - h0
                    stiles_hi = []
                    if n_hi:
                        ldh_t = ldp.tile([128, n_hi], f32, tag=f"ldh{dname}")
                        nc.sync.dma_start(out=ldh_t[:],
                                          in_=dio["ldh"][:, h0:h1])
                        for b0 in range(0, n_hi, SBATCH):
                            nb = min(SBATCH, n_hi - b0)
                            s8 = sbp.tile([128, SBATCH, 128], bf16,
                                          tag=f"sh{dname}")
                            nc.vector.tensor_tensor(
                                out=s8[:, :nb, :],
                                in0=ldh_t[:, b0:b0 + nb, None].to_broadcast(
                                    [128, nb, 128]),
                                in1=iota_t[:, None, :].to_broadcast(
                                    [128, nb, 128]),
                                op=mybir.AluOpType.is_equal)
                            stiles_hi.append(s8)

                    # host-side matmul schedule: per chunk, (col, pan, hidx)
                    # hidx None = lo stream (first cell), else hi one-hot idx
                    sched = []
                    for ch in range(NCHUNK):
                        segs = []
                        hi = {}
                        for c, pan, hg in pl["hi_seg"][(p, ch)]:
                            hi.setdefault(c, []).append((pan, hg))
                        for col, pan in enumerate(pl["lo_pan"][(p, ch)]):
                            segs.append((col, pan, None))
                            for pan2, hg in hi.get(col, ()):
                                segs.append((col, pan2, hg))
                        sched.append(segs)
                    last_seg = {}
                    for ch in range(NCHUNK):
                        for si, (_, pan, _) in enumerate(sched[ch]):
                            last_seg[pan] = (ch, si)

                    # gathers + matmuls, chunk by chunk
                    row_i = 0
                    started = set()
                    for ch in range(NCHUNK):
                        rows_c, calls = pl["struct"][p][ch]
                        tbl_chunk = table_ap[ch * CHUNK_ROWS:(ch + 1) * CHUNK_ROWS, :]
                        idx_t = idxp.tile([128, max(rows_c // 16, 8)], i16,
                                          tag=f"idx{dname}")
                        if rows_c:
                            nc.sync.dma_start(
                                out=idx_t[:, :rows_c // 16],
                                in_=dio["idx"][:, (row_off + row_i) // 16:
                                               (row_off + row_i + rows_c) // 16])
                        gtiles = []
                        r0 = 0
                        for ni in calls:
                            gt = gp.tile([128, CALL_MAX // 128, 2 * D], bf16, tag="g")
                            nc.gpsimd.dma_gather(
                                gt[:, :ni // 128, :], tbl_chunk,
                                idx_t[:, r0 // 16:(r0 + ni) // 16],
                                ni, ni, 2 * D, single_packet=SINGLE_PACKET,
                                queue_num=qrr[0] % 4)
                            qrr[0] += 1
                            gtiles.append((gt, ni))
                            r0 += ni
                        col_base = row_i // 128
                        for si, (col, pan, hidx) in enumerate(sched[ch]):
                            gi = (col * 128) // CALL_MAX
                            cic = col - gi * (CALL_MAX // 128)
                            gt, ni = gtiles[gi]
                            if hidx is None:
                                colg = col_base + col
                                s8 = stiles[colg // SBATCH]
                                rhs = s8[:, colg % SBATCH, :]
                            else:
                                hh = hidx - pl["nhi_off"][p]
                                s8 = stiles_hi[hh // SBATCH]
                                rhs = s8[:, hh % SBATCH, :]
                            lp = pan - p0
                            bank = banks[lp // 4]
                            seg = bank[:, (lp % 4) * 128:(lp % 4) * 128 + 128]
                            # start=True clears the WHOLE psum bank: only the
                            # first matmul touching a bank per (pass,
                            # direction) may set it.
                            nc.tensor.matmul(
                                seg, lhsT=gt[:, cic, :], rhs=rhs,
                                start=(lp // 4, dname) not in started,
                                stop=last_seg[pan] == (ch, si))
                            started.add((lp // 4, dname))
                        row_i += rows_c

                    # drain: mean_sb = psum * (0.5/deg), bf16, one TT per bank
                    mean_t = mean_tiles[dname]
                    for bg in range((np_this + 3) // 4):
                        w = min(512, np_this * 128 - bg * 512)
                        nc.vector.tensor_tensor(
                            out=mean_t[:, bg * 512:bg * 512 + w],
                            in0=banks[bg][:, :w],
                            in1=ivd_t[:, bg * 512:bg * 512 + w],
                            op=mybir.AluOpType.mult)

            def gemm_pass(layer, p, wl_f, wl_b, wr, bias, mean_tiles):
                """GEMMs (4 panels at a time) + relu; update hT / pool."""
                p0 = p * PANELS_PER_PASS
                np_this = min(PANELS_PER_PASS, NPANEL - p0)
                xsrc = xT_own if layer == 0 else hT_own
                for bg in range((np_this + 3) // 4):
                    w = min(512, np_this * 128 - bg * 512)
                    c0 = (p0 * 128) + bg * 512
                    gps = psu.tile([D, 512], f32, tag="gemm", name="gemmbank")
                    nc.tensor.matmul(gps[:, :w], lhsT=wl_f[:],
                                     rhs=mean_tiles["f"][:, bg * 512:bg * 512 + w],
                                     start=True, stop=False)
                    nc.tensor.matmul(gps[:, :w], lhsT=wl_b[:],
                                     rhs=mean_tiles["b"][:, bg * 512:bg * 512 + w],
                                     start=False, stop=False)
                    nc.tensor.matmul(gps[:, :w], lhsT=wr[:],
                                     rhs=xsrc[:, c0:c0 + w],
                                     start=False, stop=True)
                    hdst = hT_own
                    nc.scalar.activation(hdst[:D, c0:c0 + w], gps[:, :w],
                                         mybir.ActivationFunctionType.Relu,
                                         bias=bias[:, :1], scale=1.0)
                    # per-panel: transpose to rows; bounce (l0) or pool (l1)
                    for lpp in range(w // 128):
                        pan = p0 + bg * 4 + lpp
                        tps = pst.tile([128, 128], bf16, tag="tp")
                        nc.tensor.transpose(
                            tps[:], hdst[:, pan * 128:(pan + 1) * 128], ident[:])
                        hrow = up.tile([128, 2 * D], bf16, tag="hrow")
                        nc.vector.tensor_copy(out=hrow[:], in_=tps[:])
                        if layer == 0:
                            nc.sync.dma_start(
                                out=h_bounce[pan * 128:(pan + 1) * 128, :],
                                in_=hrow[:])
                        else:
                            mg = sbp.tile([128, G], bf16, tag="mg")
                            nc.vector.tensor_tensor(
                                out=mg[:],
                                in0=gid_t[:, pan:pan + 1].to_broadcast([128, G]),
                                in1=iota_t[:, :G],
                                op=mybir.AluOpType.is_equal)
                            nc.tensor.matmul(pool_ps[:], lhsT=hrow[:], rhs=mg[:],
                                             start=(pan == 0),
                                             stop=(pan == NPANEL - 1))

            # ---------------- layer 0 ----------------
            mean_f = mp.tile([128, PANELS_PER_PASS * 128], bf16, tag="mf")
            mean_b = mp.tile([128, PANELS_PER_PASS * 128], bf16, tag="mb")
            mean_tiles = {"f": mean_f, "b": mean_b}
            for p in range(NPASS):
                seg_pass(0, p, x_pack_d[:], mean_tiles)
                gemm_pass(0, p, wt["l0f_WlT"], wt["l0b_WlT"],
                          wt["l0_WrT"], wt["l0_bias"], mean_tiles)

            for q in range(NCHUNK):
                nc.gpsimd.collective_compute(
                    "AllGather", mybir.AluOpType.bypass,
                    replica_groups=[list(range(NCORES))],
                    ins=[h_bounce[q * QROWS:(q + 1) * QROWS, :].opt()],
                    outs=[h_table[q * NCORES * QROWS:
                                  (q + 1) * NCORES * QROWS, :].opt()])

            # ---------------- layer 1 ----------------
            for p in range(NPASS):
                seg_pass(1, p, h_table[:], mean_tiles)
                gemm_pass(1, p, wt["l1f_WlT"], wt["l1b_WlT"],
                          wt["l1_WrT"], wt["l1_bias"], mean_tiles)

            # ---------------- pool + head ----------------
            icnt_t = cp.tile([128, G], bf16)
            nc.sync.dma_start(out=icnt_t[:], in_=icnt_d[:])
            pooled = up.tile([128, G], bf16, tag="pooled")
            nc.vector.tensor_tensor(out=pooled[:], in0=pool_ps[:],
                                    in1=icnt_t[:], op=mybir.AluOpType.mult)
            ops = psu.tile([OUT, G], f32, tag="gemm")
            nc.tensor.matmul(ops[:], lhsT=wt["predWT"][:], rhs=pooled[:],
                             start=True, stop=True)
            ob = up.tile([OUT, G], f32, tag="ob")
            nc.scalar.activation(ob[:], ops[:],
                                 mybir.ActivationFunctionType.Identity,
                                 bias=wt["pred_bias"][:, :1], scale=1.0)
            nc.sync.dma_start(out=out_d[:], in_=ob[:])

    nc.compile()
    return nc


def kernel(**inputs):
    global LAST_RESULTS
    import ml_dtypes

    x = np.asarray(inputs["x"], np.float32)
    ei = np.asarray(inputs["edge_index"]).astype(np.int64)
    batch = np.asarray(inputs["batch"]).astype(np.int64)

    # ---- host planning ----
    plan_f = build_dir_plan(ei[0], ei[1])  # forward: aggregate src -> dst
    plan_b = build_dir_plan(ei[1], ei[0])  # backward
    plans = {"f": plan_f, "b": plan_b}

    deg_f = np.bincount(ei[1], minlength=N).astype(np.float32)
    deg_b = np.bincount(ei[0], minlength=N).astype(np.float32)
    ivd = {}
    for dname, dg in (("f", deg_f), ("b", deg_b)):
        v = 0.5 / np.maximum(dg, 1.0)
        vv = np.zeros((NCORES, SHARD_PAD), np.float32)
        for c in range(NCORES):
            vv[c, :SHARD] = v[c * SHARD:(c + 1) * SHARD]
        ivd[dname] = np.ascontiguousarray(
            np.broadcast_to(vv[:, None, :], (NCORES, 128, SHARD_PAD))
        ).astype(ml_dtypes.bfloat16)

    gid = np.full((NCORES, SHARD_PAD), -1.0, np.float32)
    for c in range(NCORES):
        gid[c, :SHARD] = batch[c * SHARD:(c + 1) * SHARD]
    gid_panel = gid.reshape(NCORES, NPANEL, 128).transpose(0, 2, 1).copy()

    cnt = np.bincount(batch, minlength=G).astype(np.float32)
    inv_cnt = (1.0 / np.maximum(cnt, 1.0)).astype(np.float32)
    inv_cnt_exp = np.ascontiguousarray(
        np.broadcast_to(inv_cnt[None, :], (128, G))).astype(ml_dtypes.bfloat16)

    # packed tables (quarter-major row mapping, see tbl_row)
    xp = np.zeros((TBL_ROWS, D), np.float32)
    xp[tbl_row(np.arange(N))] = x
    x_pack = pack_hilo(xp)
    # transposed packed own-features [core, 128, SHARD_PAD] (loc-ordered)
    xT = []
    for c in range(NCORES):
        xc = np.zeros((SHARD_PAD, D), np.float32)
        xc[:SHARD] = x[c * SHARD:(c + 1) * SHARD]
        xT.append(np.ascontiguousarray(pack_hilo(xc).T))
    xT = np.stack(xT)

    def st(a):  # hi|lo stack for packed rhs
        aT = np.asarray(a, np.float32).T
        return np.ascontiguousarray(np.vstack([aT, aT])).astype(ml_dtypes.bfloat16)

    w = {
        "l0f_WlT": st(inputs["l0f_Wl"]),
        "l0b_WlT": st(inputs["l0b_Wl"]),
        "l0_WrT": st(0.5 * (np.asarray(inputs["l0f_Wr"], np.float32)
                            + np.asarray(inputs["l0b_Wr"], np.float32))),
        "l0_bias": np.ascontiguousarray(
            0.5 * (np.asarray(inputs["l0f_bl"], np.float32)
                   + np.asarray(inputs["l0b_bl"], np.float32))[:, None]),
        "l1f_WlT": st(inputs["l1f_Wl"]),
        "l1b_WlT": st(inputs["l1b_Wl"]),
        "l1_WrT": st(0.5 * (np.asarray(inputs["l1f_Wr"], np.float32)
                            + np.asarray(inputs["l1b_Wr"], np.float32))),
        "l1_bias": np.ascontiguousarray(
            0.5 * (np.asarray(inputs["l1f_bl"], np.float32)
                   + np.asarray(inputs["l1b_bl"], np.float32))[:, None]),
        "predWT": st(inputs["pred_W"]),
        "pred_bias": np.ascontiguousarray(
            (np.asarray(inputs["pred_b"], np.float32) / NCORES)[:, None]),
    }

    nc = build_bass(plans, w)

    iota = np.ascontiguousarray(
        np.broadcast_to(np.arange(128, dtype=np.float32)[None, :], (128, 128)))
    in_maps = []
    for c in range(NCORES):
        m = {"x_pack": x_pack, "iota": iota, "xT": xT[c],
             "idx_f": plan_f["idx"][c], "ld_f": plan_f["ld"][c],
             "idx_b": plan_b["idx"][c], "ld_b": plan_b["ld"][c],
             "ldh_f": plan_f["ldh"][c], "ldh_b": plan_b["ldh"][c],
             "ivd_f": ivd["f"][c], "ivd_b": ivd["b"][c],
             "gid": gid_panel[c], "icnt": inv_cnt_exp}
        m.update(w)
        in_maps.append(m)

    res = run_bass_kernel_spmd(
        nc, in_maps, core_ids=list(range(NCORES)),
        trace=bool(os.environ.get("BASS_TRACE")))
    LAST_RESULTS = res
    out = np.zeros((OUT, G), np.float32)
    for c in range(NCORES):
        out += np.asarray(res.results[c]["out"], np.float32)
    return np.ascontiguousarray(out.T)
